# revision 6
# baseline (speedup 1.0000x reference)
"""Trainium2 Bass kernel for nn_Attention_Decoder (bs=512, T=1024, H=256).

Sharding: data-parallel over batch across 8 NeuronCores (64 batches/core).
enc_vec is uploaded twice in bf16: transposed [h, t] layout (for the big
W1 @ enc matmul) and natural [t, h] layout (for the softmax-weighted sum).
All weights are pre-transposed on the host into lhsT block layout.
"""

import os

import numpy as np
import ml_dtypes

import concourse.bacc as bacc
import concourse.tile as tile
from concourse import mybir, bass_utils

BF16 = ml_dtypes.bfloat16
F32 = np.float32

H = 256
T = 1024
BS = 512
NCORES = 8
NMEL, RF = 80, 5
MOUT = NMEL * RF  # 400

dt = mybir.dt
AF = mybir.ActivationFunctionType
AX = mybir.AxisListType

_CACHE = {}


# --------------------------------------------------------------------------
# Device program
# --------------------------------------------------------------------------

def _build_nc(B):
    """Build + compile the per-core Bass program for B batches per core."""
    nc = bacc.Bacc("TRN2", target_bir_lowering=False, debug=False)

    f32, bf16 = dt.float32, dt.bfloat16
    d = {}

    def inp(name, shape, dtype):
        d[name] = nc.dram_tensor(name, shape, dtype, kind="ExternalInput")

    def outp(name, shape, dtype):
        d[name] = nc.dram_tensor(name, shape, dtype, kind="ExternalOutput")

    inp("encT", [B, 128, 2048], bf16)
    inp("encN", [B, 128, 2048], bf16)
    inp("decT", [128, B], f32)
    inp("hA", [128, 2 * B], f32)
    inp("h1", [128, 2 * B], f32)
    inp("h2", [128, 2 * B], f32)
    inp("w1", [128, 512], bf16)
    inp("v", [128, 2], bf16)
    inp("bS", [128, 2], f32)
    inp("w2", [128, 512], f32)
    inp("aw_ih", [128, 768], f32)
    inp("aw_hh", [128, 1536], f32)
    inp("a_brz", [128, 4], f32)
    inp("a_bin", [128, 2], f32)
    inp("a_bhn", [128, 2], f32)
    inp("g1_ih", [128, 1536], f32)
    inp("g1_hh", [128, 1536], f32)
    inp("g1_brz", [128, 4], f32)
    inp("g1_bin", [128, 2], f32)
    inp("g1_bhn", [128, 2], f32)
    inp("g2_ih", [128, 1536], f32)
    inp("g2_hh", [128, 1536], f32)
    inp("g2_brz", [128, 4], f32)
    inp("g2_bin", [128, 2], f32)
    inp("g2_bhn", [128, 2], f32)
    inp("pw", [128, 1024], f32)
    inp("pb", [128, 2], f32)
    inp("ow", [128, 800], f32)
    inp("ob", [128, 4], f32)
    inp("onc", [128, 1], f32)
    inp("onr", [1, 128], f32)

    outp("o_mel", [128, 4 * B], f32)
    outp("o_hA", [128, 2 * B], f32)
    outp("o_h1", [128, 2 * B], f32)
    outp("o_h2", [128, 2 * B], f32)

    with tile.TileContext(nc) as tc:
        _emit(nc, tc, B, d)
    nc.compile()
    return nc


def _emit(nc, tc, B, d):
    f32, bf16 = dt.float32, dt.bfloat16
    from contextlib import ExitStack

    with ExitStack() as ctx:
        consts = ctx.enter_context(tc.tile_pool(name="consts", bufs=1))
        acc = ctx.enter_context(tc.tile_pool(name="acc", bufs=1))
        sm = ctx.enter_context(tc.tile_pool(name="sm", bufs=1))
        ps_main = ctx.enter_context(tc.tile_pool(name="ps_main", bufs=5, space="PSUM"))
        ps_uq = ctx.enter_context(tc.tile_pool(name="ps_uq", bufs=1, space="PSUM"))
        ps_dq = ctx.enter_context(tc.tile_pool(name="ps_dq", bufs=1, space="PSUM"))
        p_encT = ctx.enter_context(tc.tile_pool(name="p_encT", bufs=3))
        p_encN = ctx.enter_context(tc.tile_pool(name="p_encN", bufs=3))
        p_tanh = ctx.enter_context(tc.tile_pool(name="p_tanh", bufs=2))

        def cload(name, shape, dtype):
            t = consts.tile(shape, dtype, tag=name, name=f"sb_{name}")
            nc.sync.dma_start(t[:], d[name].ap()[:])
            return t

        w1_sb = cload("w1", [128, 512], bf16)
        v_sb = cload("v", [128, 2], bf16)
        bS_sb = cload("bS", [128, 2], f32)
        w2_sb = cload("w2", [128, 512], f32)
        decT_sb = cload("decT", [128, B], f32)
        hA_sb = cload("hA", [128, 2 * B], f32)
        h1_sb = cload("h1", [128, 2 * B], f32)
        h2_sb = cload("h2", [128, 2 * B], f32)
        aw_ih_sb = cload("aw_ih", [128, 768], f32)
        aw_hh_sb = cload("aw_hh", [128, 1536], f32)
        a_brz_sb = cload("a_brz", [128, 4], f32)
        a_bin_sb = cload("a_bin", [128, 2], f32)
        a_bhn_sb = cload("a_bhn", [128, 2], f32)
        g1_ih_sb = cload("g1_ih", [128, 1536], f32)
        g1_hh_sb = cload("g1_hh", [128, 1536], f32)
        g1_brz_sb = cload("g1_brz", [128, 4], f32)
        g1_bin_sb = cload("g1_bin", [128, 2], f32)
        g1_bhn_sb = cload("g1_bhn", [128, 2], f32)
        g2_ih_sb = cload("g2_ih", [128, 1536], f32)
        g2_hh_sb = cload("g2_hh", [128, 1536], f32)
        g2_brz_sb = cload("g2_brz", [128, 4], f32)
        g2_bin_sb = cload("g2_bin", [128, 2], f32)
        g2_bhn_sb = cload("g2_bhn", [128, 2], f32)
        pw_sb = cload("pw", [128, 1024], f32)
        pb_sb = cload("pb", [128, 2], f32)
        ow_sb = cload("ow", [128, 800], f32)
        ob_sb = cload("ob", [128, 4], f32)
        onc_sb = cload("onc", [128, 1], f32)
        onr_sb = cload("onr", [1, 128], f32)

        encT_ap = d["encT"].ap()
        encN_ap = d["encN"].ap()

        def gru(name, x_blocks, h_sb_ap, wih_sb, whh_sb, ktx, brz, bin_, bhn,
                out_pool):
            """One GRU step on transposed activations [feat, B]. h_sb_ap is a
            [128, 2B] tile; returns two [128, B] hout tiles from out_pool."""
            rz = []
            for gb in range(4):
                ps_g = ps_main.tile([128, 512], f32, tag="s", name=f"{name}_psg{gb}")
                nmm = ktx + 2
                i = 0
                for kt in range(ktx):
                    nc.tensor.matmul(
                        ps_g[:, 0:B],
                        wih_sb[:, kt * 768 + gb * 128: kt * 768 + gb * 128 + 128],
                        x_blocks[kt][:],
                        start=(i == 0), stop=(i == nmm - 1))
                    i += 1
                for kt in range(2):
                    nc.tensor.matmul(
                        ps_g[:, 0:B],
                        whh_sb[:, kt * 768 + gb * 128: kt * 768 + gb * 128 + 128],
                        h_sb_ap[:, kt * B:(kt + 1) * B],
                        start=(i == 0), stop=(i == nmm - 1))
                    i += 1
                g_sb = sm.tile([128, B], f32, tag=f"{name}_g{gb}",
                               name=f"{name}_g{gb}")
                nc.scalar.activation(g_sb[:], ps_g[:, 0:B], AF.Sigmoid,
                                     bias=brz[:, gb:gb + 1])
                rz.append(g_sb)
            houts = []
            for nb in range(2):
                gb = 4 + nb
                ps_i = ps_main.tile([128, 512], f32, tag="s", name=f"{name}_psi{nb}")
                for kt in range(ktx):
                    nc.tensor.matmul(
                        ps_i[:, 0:B],
                        wih_sb[:, kt * 768 + gb * 128: kt * 768 + gb * 128 + 128],
                        x_blocks[kt][:],
                        start=(kt == 0), stop=(kt == ktx - 1))
                ps_h = ps_main.tile([128, 512], f32, tag="s", name=f"{name}_psh{nb}")
                for kt in range(2):
                    nc.tensor.matmul(
                        ps_h[:, 0:B],
                        whh_sb[:, kt * 768 + gb * 128: kt * 768 + gb * 128 + 128],
                        h_sb_ap[:, kt * B:(kt + 1) * B],
                        start=(kt == 0), stop=(kt == 1))
                hnb = sm.tile([128, B], f32, tag=f"{name}_hnb{nb}",
                              name=f"{name}_hnb{nb}")
                nc.scalar.activation(hnb[:], ps_h[:, 0:B], AF.Identity,
                                     bias=bhn[:, nb:nb + 1])
                rhn = sm.tile([128, B], f32, tag=f"{name}_rhn{nb}",
                              name=f"{name}_rhn{nb}")
                nc.vector.tensor_mul(rhn[:], rz[nb][:], hnb[:])
                t1 = sm.tile([128, B], f32, tag=f"{name}_t1{nb}",
                             name=f"{name}_t1{nb}")
                nc.vector.tensor_add(t1[:], ps_i[:, 0:B], rhn[:])
                n_sb = sm.tile([128, B], f32, tag=f"{name}_n{nb}",
                               name=f"{name}_n{nb}")
                nc.scalar.activation(n_sb[:], t1[:], AF.Tanh,
                                     bias=bin_[:, nb:nb + 1])
                dd = sm.tile([128, B], f32, tag=f"{name}_d{nb}",
                             name=f"{name}_d{nb}")
                nc.vector.tensor_sub(dd[:], h_sb_ap[:, nb * B:(nb + 1) * B], n_sb[:])
                zd = sm.tile([128, B], f32, tag=f"{name}_zd{nb}",
                             name=f"{name}_zd{nb}")
                nc.vector.tensor_mul(zd[:], rz[2 + nb][:], dd[:])
                ho = out_pool.tile([128, B], f32, tag=f"{name}_h{nb}",
                                   name=f"{name}_h{nb}")
                nc.vector.tensor_add(ho[:], n_sb[:], zd[:])
                houts.append(ho)
            return houts

        # ---------------- prologue: attention GRU + score bias ------------
        dtT = gru("ga", [decT_sb], hA_sb, aw_ih_sb, aw_hh_sb, 1,
                  a_brz_sb, a_bin_sb, a_bhn_sb, consts)
        for hb in range(2):
            nc.sync.dma_start(d["o_hA"].ap()[:, hb * B:(hb + 1) * B], dtT[hb][:])

        biasS_sb = consts.tile([128, 2 * B], f32, tag="biasS", name="biasS_sb")
        for hb in range(2):
            ps_bs = ps_main.tile([128, 512], f32, tag="s", name=f"ps_bs{hb}")
            for kt in range(2):
                nc.tensor.matmul(
                    ps_bs[:, 0:B],
                    w2_sb[:, kt * 256 + hb * 128: kt * 256 + hb * 128 + 128],
                    dtT[kt][:],
                    start=(kt == 0), stop=(kt == 1))
            nc.scalar.activation(biasS_sb[:, hb * B:(hb + 1) * B], ps_bs[:, 0:B],
                                 AF.Identity, bias=bS_sb[:, hb:hb + 1])

        # ---------------- main streaming loop over batches ----------------
        e_sb = acc.tile([128, 8 * B], bf16, tag="e", name="e_sb")
        zp_sb = acc.tile([128, B], f32, tag="zp", name="zp_sb")
        ps_u = [ps_uq.tile([128, 256], f32, tag=f"u{i}", name=f"ps_u{i}")
                for i in range(2)]
        ps_d = ps_dq.tile([128, 128], f32, tag="d", name="ps_d")

        tanh_tiles = {}
        encN_tiles = {}
        for i in range(B + 2):
            if i < B:
                b = i
                encT_sb = p_encT.tile([128, 2048], bf16, tag="encT",
                                      name=f"encT{b}")
                nc.sync.dma_start(encT_sb[:], encT_ap[b])
                encN_sb = p_encN.tile([128, 2048], bf16, tag="encN",
                                      name=f"encN{b}")
                nc.sync.dma_start(encN_sb[:], encN_ap[b])
                encN_tiles[b] = encN_sb
                th = p_tanh.tile([128, 2048], bf16, tag="tanh", name=f"tanh{b}")
                tanh_tiles[b] = th
                for t2 in range(2):
                    for hb in range(2):
                        ps_s = ps_main.tile([128, 512], f32, tag="s",
                                            name=f"psS{b}_{t2}_{hb}")
                        for kt in range(2):
                            nc.tensor.matmul(
                                ps_s[:],
                                w1_sb[:, kt * 256 + hb * 128:
                                      kt * 256 + hb * 128 + 128],
                                encT_sb[:, kt * 1024 + t2 * 512:
                                        kt * 1024 + t2 * 512 + 512],
                                start=(kt == 0), stop=(kt == 1))
                        nc.scalar.activation(
                            th[:, hb * 1024 + t2 * 512: hb * 1024 + t2 * 512 + 512],
                            ps_s[:], AF.Tanh,
                            bias=biasS_sb[:, hb * B + b: hb * B + b + 1])
            if 1 <= i <= B:
                b = i - 1
                th = tanh_tiles.pop(b)
                pu = ps_u[b % 2]
                base = (b // 2) * 8
                for k in range(8):
                    for hb in range(2):
                        nc.tensor.matmul(
                            pu[:, base + k: base + k + 1],
                            th[:, hb * 1024 + k * 128: hb * 1024 + k * 128 + 128],
                            v_sb[:, hb:hb + 1],
                            start=(hb == 0), stop=(hb == 1))
                nc.scalar.activation(e_sb[:, b * 8:(b + 1) * 8],
                                     pu[:, base:base + 8], AF.Exp)
                nc.vector.reduce_sum(zp_sb[:, b:b + 1], e_sb[:, b * 8:b * 8 + 8],
                                     axis=AX.X)
            if 2 <= i <= B + 1:
                b = i - 2
                encN_sb = encN_tiles.pop(b)
                for hb in range(2):
                    for k in range(8):
                        nc.tensor.matmul(
                            ps_d[:, hb * B + b: hb * B + b + 1],
                            encN_sb[:, k * 256 + hb * 128: k * 256 + hb * 128 + 128],
                            e_sb[:, b * 8 + k: b * 8 + k + 1],
                            start=(k == 0), stop=(k == 7))

        # ---------------- softmax normalization of d_t_dot ----------------
        ps_z = ps_main.tile([128, 512], f32, tag="s", name="ps_z")
        nc.tensor.matmul(ps_z[0:1, 0:B], onc_sb[:], zp_sb[:],
                         start=True, stop=True)
        rZ_sb = sm.tile([1, B], f32, tag="rZ", name="rZ_sb")
        nc.vector.reciprocal(rZ_sb[:], ps_z[0:1, 0:B])
        ps_r = ps_main.tile([128, 512], f32, tag="s", name="ps_r")
        nc.tensor.matmul(ps_r[:, 0:B], onr_sb[:], rZ_sb[:], start=True, stop=True)
        rrep_sb = sm.tile([128, B], f32, tag="rrep", name="rrep_sb")
        nc.scalar.copy(rrep_sb[:], ps_r[:, 0:B])
        dtd = []
        for hb in range(2):
            dt_t = consts.tile([128, B], f32, tag=f"dtd{hb}", name=f"dtd{hb}")
            nc.vector.tensor_mul(dt_t[:], ps_d[:, hb * B:(hb + 1) * B], rrep_sb[:])
            dtd.append(dt_t)

        # ---------------- projection ----------------
        concat_blocks = [dtT[0], dtT[1], dtd[0], dtd[1]]
        projT = []
        for mb in range(2):
            ps_p = ps_main.tile([128, 512], f32, tag="s", name=f"ps_p{mb}")
            for kt in range(4):
                nc.tensor.matmul(
                    ps_p[:, 0:B],
                    pw_sb[:, kt * 256 + mb * 128: kt * 256 + mb * 128 + 128],
                    concat_blocks[kt][:],
                    start=(kt == 0), stop=(kt == 3))
            pj = consts.tile([128, B], f32, tag=f"projT{mb}", name=f"projT{mb}")
            nc.scalar.activation(pj[:], ps_p[:, 0:B], AF.Identity,
                                 bias=pb_sb[:, mb:mb + 1])
            projT.append(pj)

        # ---------------- GRU1 / GRU2 / output ----------------
        g1out = gru("g1", projT, h1_sb, g1_ih_sb, g1_hh_sb, 2,
                    g1_brz_sb, g1_bin_sb, g1_bhn_sb, consts)
        for hb in range(2):
            nc.sync.dma_start(d["o_h1"].ap()[:, hb * B:(hb + 1) * B], g1out[hb][:])

        in2 = []
        for hb in range(2):
            t = consts.tile([128, B], f32, tag=f"in2_{hb}", name=f"in2_{hb}")
            nc.vector.tensor_add(t[:], g1out[hb][:], projT[hb][:])
            in2.append(t)

        g2out = gru("g2", in2, h2_sb, g2_ih_sb, g2_hh_sb, 2,
                    g2_brz_sb, g2_bin_sb, g2_bhn_sb, consts)
        for hb in range(2):
            nc.sync.dma_start(d["o_h2"].ap()[:, hb * B:(hb + 1) * B], g2out[hb][:])

        sum2 = []
        for hb in range(2):
            t = consts.tile([128, B], f32, tag=f"sum2_{hb}", name=f"sum2_{hb}")
            nc.vector.tensor_add(t[:], in2[hb][:], g2out[hb][:])
            sum2.append(t)

        for mb in range(4):
            mw = min(128, MOUT - mb * 128)
            ps_o = ps_main.tile([128, 512], f32, tag="s", name=f"ps_o{mb}")
            for kt in range(2):
                nc.tensor.matmul(
                    ps_o[0:mw, 0:B],
                    ow_sb[:, kt * MOUT + mb * 128: kt * MOUT + mb * 128 + mw],
                    sum2[kt][:],
                    start=(kt == 0), stop=(kt == 1))
            strip = sm.tile([128, B], f32, tag="sm_o", bufs=2, name=f"mel{mb}")
            nc.scalar.activation(strip[0:mw, :], ps_o[0:mw, 0:B], AF.Identity,
                                 bias=ob_sb[0:mw, mb:mb + 1])
            nc.sync.dma_start(d["o_mel"].ap()[0:mw, mb * B:(mb + 1) * B],
                              strip[0:mw, :])


# --------------------------------------------------------------------------
# Host side
# --------------------------------------------------------------------------

def _pblk(x):
    """[K, F] -> [128, (K//128)*F] partition-major block layout."""
    x = np.ascontiguousarray(x)
    K, F = x.shape
    Kt = K // 128
    return np.ascontiguousarray(
        x.reshape(Kt, 128, F).transpose(1, 0, 2)).reshape(128, Kt * F)


def _prep_shared(inp):
    """Weight tensors (replicated to every core)."""
    g = lambda k: np.asarray(inp[k], dtype=F32)
    out = {}
    out["w1"] = _pblk(g("W1_w").T).astype(BF16)
    out["v"] = np.ascontiguousarray(g("V_w").reshape(2, 128).T).astype(BF16)
    out["bS"] = np.ascontiguousarray((g("W1_b") + g("W2_b")).reshape(2, 128).T)
    out["w2"] = _pblk(g("W2_w").T)
    for pfx, wih, whh, bih, bhh in (
            ("a", "agru_wih", "agru_whh", "agru_bih", "agru_bhh"),
            ("g1", "gru1_wih", "gru1_whh", "gru1_bih", "gru1_bhh"),
            ("g2", "gru2_wih", "gru2_whh", "gru2_bih", "gru2_bhh")):
        k_ih = "aw_ih" if pfx == "a" else f"{pfx}_ih"
        k_hh = "aw_hh" if pfx == "a" else f"{pfx}_hh"
        out[k_ih] = _pblk(g(wih).T)
        out[k_hh] = _pblk(g(whh).T)
        bih_v, bhh_v = g(bih), g(bhh)
        out[f"{pfx}_brz"] = np.ascontiguousarray(
            (bih_v + bhh_v)[:512].reshape(4, 128).T)
        out[f"{pfx}_bin"] = np.ascontiguousarray(bih_v[512:].reshape(2, 128).T)
        out[f"{pfx}_bhn"] = np.ascontiguousarray(bhh_v[512:].reshape(2, 128).T)
    out["pw"] = _pblk(g("proj_w").T)
    out["pb"] = np.ascontiguousarray(g("proj_b").reshape(2, 128).T)
    out["ow"] = _pblk(g("out_w").T)
    ob = np.zeros(512, dtype=F32)
    ob[:MOUT] = g("out_b")
    out["ob"] = np.ascontiguousarray(ob.reshape(4, 128).T)
    out["onc"] = np.ones([128, 1], dtype=F32)
    out["onr"] = np.ones([1, 128], dtype=F32)
    return out


def _make_in_maps(inputs, ncores, B):
    enc = np.asarray(inputs["enc_vec"], dtype=F32)
    dec = np.asarray(inputs["dec_vec"], dtype=F32)
    hA = np.asarray(inputs["atten_GRU_h"], dtype=F32)
    h1 = np.asarray(inputs["gru1_h"], dtype=F32)
    h2 = np.asarray(inputs["gru2_h"], dtype=F32)

    shared = _prep_shared(inputs)
    in_maps = []
    for c in range(ncores):
        sl = slice(c * B, (c + 1) * B)
        encs = enc[sl]
        encT = encs.transpose(0, 2, 1).reshape(B, 2, 128, T) \
            .transpose(0, 2, 1, 3).reshape(B, 128, 2048).astype(BF16)
        encN = encs.reshape(B, 8, 128, H).transpose(0, 2, 1, 3) \
            .reshape(B, 128, 2048).astype(BF16)
        m = dict(shared)
        m["encT"] = np.ascontiguousarray(encT)
        m["encN"] = np.ascontiguousarray(encN)
        m["decT"] = np.ascontiguousarray(dec[sl, 0, :].T)
        m["hA"] = _pblk(hA[0, sl].T)
        m["h1"] = _pblk(h1[0, sl].T)
        m["h2"] = _pblk(h2[0, sl].T)
        in_maps.append(m)
    return in_maps


def _assemble(results, ncores, B):
    outs, hAs, h1s, h2s = [], [], [], []
    for c in range(ncores):
        r = results[c]
        mel = np.asarray(r["o_mel"], dtype=F32)  # [128, 4B]
        mel = mel.reshape(128, 4, B).transpose(1, 0, 2).reshape(512, B)
        outs.append(mel[:MOUT].T.reshape(B, RF, NMEL))

        def unb(x):  # [128, 2B] -> [B, 256]
            x = np.asarray(x, dtype=F32)
            return x.reshape(128, 2, B).transpose(1, 0, 2).reshape(256, B).T

        hAs.append(unb(r["o_hA"]))
        h1s.append(unb(r["o_h1"]))
        h2s.append(unb(r["o_h2"]))

    out = np.concatenate(outs, axis=0)
    next_hA = np.concatenate(hAs, axis=0)[None]
    next_h1 = np.concatenate(h1s, axis=0)[None]
    next_h2 = np.concatenate(h2s, axis=0)[None]
    return out, next_hA, next_h1, next_h2


def kernel(**inputs):
    B = BS // NCORES
    if "nc" not in _CACHE:
        _CACHE["nc"] = _build_nc(B)
    nc = _CACHE["nc"]

    in_maps = _make_in_maps(inputs, NCORES, B)
    trace = os.environ.get("KERNEL_TRACE") == "1"
    res = bass_utils.run_bass_kernel_spmd(
        nc, in_maps, core_ids=list(range(NCORES)), trace=trace)
    _CACHE["last_results"] = res
    return _assemble(res.results, NCORES, B)


# revision 36
# speedup vs baseline: 25.8237x; 25.8237x over previous
"""Trainium2 Bass kernel for nn_Attention_Decoder (bs=512, T=1024, H=256).

Sharding: data-parallel over batch across 8 NeuronCores (64 batches/core).
enc_vec is uploaded twice: transposed [h, t] layout in bf16 (for the big
W1 @ enc matmul) and natural [t, h] layout in fp8e4m3 (weights of the
softmax-weighted-sum matmuls). All weights are pre-transposed on the host
into lhsT block layout.
"""

import os

import numpy as np
import ml_dtypes

import concourse.bacc as bacc
import concourse.tile as tile
from concourse import mybir, bass_utils

BF16 = ml_dtypes.bfloat16
FP8 = ml_dtypes.float8_e4m3
F32 = np.float32

H = 256
T = 1024
BS = 512
NCORES = 8
NMEL, RF = 80, 5
MOUT = NMEL * RF  # 400
ENCN_FP8 = True
ENCT_FP8 = True
CHUNK = 4  # batches per enc DMA

dt = mybir.dt
AF = mybir.ActivationFunctionType
AX = mybir.AxisListType

_CACHE = {}


def _pro_layout(B):
    """Prologue-critical f32 constants (needed in the first ~10us)."""
    return [
        ("bS", 2), ("w2", 512), ("decT", B), ("hA", 2 * B), ("aw_ih", 768),
        ("aw_hh", 1536), ("a_brz", 4), ("a_bin", 2), ("a_bhn", 2),
        ("ones", 128),
    ]


def _epi_layout(B):
    """Epilogue f32 constants (needed only after the main loop)."""
    return [
        ("h1", 2 * B), ("h2", 2 * B), ("g1_ih", 1536), ("g1_hh", 1536),
        ("g1_brz", 4), ("g1_bin", 2), ("g1_bhn", 2), ("g2_ih", 1536),
        ("g2_hh", 1536), ("g2_brz", 4), ("g2_bin", 2), ("g2_bhn", 2),
        ("pw", 1024), ("pb", 2), ("ow", 800), ("ob", 4),
    ]


def _offsets(layout):
    off, o = {}, 0
    for name, w in layout:
        off[name] = o
        o += w
    return off, o


def _encn_dt():
    return dt.float8e4 if ENCN_FP8 else dt.bfloat16


def _encn_np():
    return FP8 if ENCN_FP8 else BF16


def _enct_dt():
    return dt.float8e4 if ENCT_FP8 else dt.bfloat16


def _enct_np():
    return FP8 if ENCT_FP8 else BF16


# --------------------------------------------------------------------------
# Device program
# --------------------------------------------------------------------------

def _build_nc(B):
    """Build + compile the per-core Bass program for B batches per core."""
    nc = bacc.Bacc("TRN2", target_bir_lowering=False, debug=False)

    f32, bf16 = dt.float32, dt.bfloat16
    d = {}

    def inp(name, shape, dtype):
        d[name] = nc.dram_tensor(name, shape, dtype, kind="ExternalInput")

    def outp(name, shape, dtype):
        d[name] = nc.dram_tensor(name, shape, dtype, kind="ExternalOutput")

    _, protot = _offsets(_pro_layout(B))
    _, epitot = _offsets(_epi_layout(B))
    inp("encT", [B, 128, 2048], _enct_dt())
    inp("encN", [B, 128, 2048], _encn_dt())
    inp("cpro", [128, protot], f32)
    inp("cepi", [128, epitot], f32)
    inp("cb16", [128, 514], bf16)
    inp("cw8", [128, 512], _enct_dt())

    outp("o_mel", [128, 4 * B], f32)
    outp("o_hA", [128, 2 * B], f32)
    outp("o_h1", [128, 2 * B], f32)
    outp("o_h2", [128, 2 * B], f32)

    with tile.TileContext(nc) as tc:
        _emit(nc, tc, B, d)
    nc.compile()
    return nc


def _gru(nc, tc, B, name, x_blocks, h_sb_ap, wih_sb, whh_sb, ktx,
         brz, bin_, bhn, ps_pool, sm, out_pool):
    """One GRU step on transposed activations [feat, B]. h_sb_ap is a
    [128, 2B] tile; returns two [128, B] hout tiles from out_pool."""
    f32 = dt.float32
    rz = []
    for gb in range(4):
        ps_g = ps_pool.tile([128, 512], f32, tag="g", name=f"{name}_psg{gb}")
        nmm = ktx + 2
        i = 0
        for kt in range(ktx):
            nc.tensor.matmul(
                ps_g[:, 0:B],
                wih_sb[:, kt * 768 + gb * 128: kt * 768 + gb * 128 + 128],
                x_blocks[kt][:],
                start=(i == 0), stop=(i == nmm - 1))
            i += 1
        for kt in range(2):
            nc.tensor.matmul(
                ps_g[:, 0:B],
                whh_sb[:, kt * 768 + gb * 128: kt * 768 + gb * 128 + 128],
                h_sb_ap[:, kt * B:(kt + 1) * B],
                start=(i == 0), stop=(i == nmm - 1))
            i += 1
        g_sb = sm.tile([128, B], f32, tag=f"{name}_g{gb}", name=f"{name}_g{gb}")
        nc.scalar.activation(g_sb[:], ps_g[:, 0:B], AF.Sigmoid,
                             bias=brz[:, gb:gb + 1])
        rz.append(g_sb)
    houts = []
    for nb in range(2):
        gb = 4 + nb
        ps_i = ps_pool.tile([128, 512], f32, tag="g", name=f"{name}_psi{nb}")
        for kt in range(ktx):
            nc.tensor.matmul(
                ps_i[:, 0:B],
                wih_sb[:, kt * 768 + gb * 128: kt * 768 + gb * 128 + 128],
                x_blocks[kt][:],
                start=(kt == 0), stop=(kt == ktx - 1))
        ps_h = ps_pool.tile([128, 512], f32, tag="g", name=f"{name}_psh{nb}")
        for kt in range(2):
            nc.tensor.matmul(
                ps_h[:, 0:B],
                whh_sb[:, kt * 768 + gb * 128: kt * 768 + gb * 128 + 128],
                h_sb_ap[:, kt * B:(kt + 1) * B],
                start=(kt == 0), stop=(kt == 1))
        hnb = sm.tile([128, B], f32, tag=f"{name}_hnb{nb}", name=f"{name}_hnb{nb}")
        nc.vector.tensor_scalar_add(hnb[:], ps_h[:, 0:B], bhn[:, nb:nb + 1])
        rhn = sm.tile([128, B], f32, tag=f"{name}_rhn{nb}", name=f"{name}_rhn{nb}")
        nc.vector.tensor_mul(rhn[:], rz[nb][:], hnb[:])
        t1 = sm.tile([128, B], f32, tag=f"{name}_t1{nb}", name=f"{name}_t1{nb}")
        nc.vector.tensor_add(t1[:], ps_i[:, 0:B], rhn[:])
        n_sb = sm.tile([128, B], f32, tag=f"{name}_n{nb}", name=f"{name}_n{nb}")
        nc.scalar.activation(n_sb[:], t1[:], AF.Tanh, bias=bin_[:, nb:nb + 1])
        dd = sm.tile([128, B], f32, tag=f"{name}_d{nb}", name=f"{name}_d{nb}")
        nc.vector.tensor_sub(dd[:], h_sb_ap[:, nb * B:(nb + 1) * B], n_sb[:])
        zd = sm.tile([128, B], f32, tag=f"{name}_zd{nb}", name=f"{name}_zd{nb}")
        nc.vector.tensor_mul(zd[:], rz[2 + nb][:], dd[:])
        ho = out_pool.tile([128, B], f32, tag=f"{name}_h{nb}", name=f"{name}_h{nb}")
        nc.vector.tensor_add(ho[:], n_sb[:], zd[:])
        houts.append(ho)
    return houts


def _emit(nc, tc, B, d):
    f32, bf16 = dt.float32, dt.bfloat16
    endt = _encn_dt()
    from contextlib import ExitStack

    assert B % (2 * CHUNK) == 0
    nchunks = B // CHUNK

    with ExitStack() as ctx:
        consts = ctx.enter_context(tc.tile_pool(name="consts", bufs=1))
        acc = ctx.enter_context(tc.tile_pool(name="acc", bufs=1))
        sm = ctx.enter_context(tc.tile_pool(name="sm", bufs=1))
        p_encT = ctx.enter_context(tc.tile_pool(name="p_encT", bufs=5))
        p_encN = ctx.enter_context(tc.tile_pool(name="p_encN", bufs=5))
        p_tanh = ctx.enter_context(tc.tile_pool(name="p_tanh", bufs=3))
        # PSUM pools: ps_dq (1 bank) lives from main loop into the epilogue.
        ps_dq = ctx.enter_context(tc.tile_pool(name="ps_dq", bufs=1, space="PSUM"))

        # hoist the first enc chunk DMAs ahead of everything else so the
        # tensor engine has work as soon as possible
        encT_ap0 = d["encT"].ap()
        encN_ap0 = d["encN"].ap()
        encT_tiles = {}
        encN_tiles = {}

        def load_chunk(c):
            eT = p_encT.tile([128, CHUNK * 2048], _enct_dt(), tag="encT",
                             name=f"encTc{c}")
            nc.sync.dma_start(
                eT[:],
                encT_ap0[c * CHUNK:(c + 1) * CHUNK].rearrange("b p x -> p b x"))
            encT_tiles[c] = eT
            eN = p_encN.tile([128, CHUNK * 2048], _encn_dt(), tag="encN",
                             name=f"encNc{c}")
            nc.gpsimd.dma_start(
                eN[:],
                encN_ap0[c * CHUNK:(c + 1) * CHUNK].rearrange("b p x -> p b x"))
            encN_tiles[c] = eN

        # packed consts: prologue blob first (small), epilogue blob later
        opro, protot = _offsets(_pro_layout(B))
        oepi, epitot = _offsets(_epi_layout(B))
        cpro_sb = consts.tile([128, protot], dt.float32, tag="cpro",
                              name="cpro_sb")
        nc.gpsimd.dma_start(cpro_sb[:], d["cpro"].ap()[:])
        cb16_sb = consts.tile([128, 514], bf16, tag="cb16", name="cb16_sb")
        nc.gpsimd.dma_start(cb16_sb[:], d["cb16"].ap()[:])
        cw8_sb = consts.tile([128, 512], _enct_dt(), tag="cw8", name="cw8_sb")
        nc.gpsimd.dma_start(cw8_sb[:], d["cw8"].ap()[:])

        load_chunk(0)
        load_chunk(1)
        load_chunk(2)
        load_chunk(3)

        # epilogue consts: big blob, needed only ~150us in
        cepi_sb = consts.tile([128, epitot], dt.float32, tag="cepi",
                              name="cepi_sb")
        nc.gpsimd.dma_start(cepi_sb[:], d["cepi"].ap()[:])

        def cp(name, width):
            return cpro_sb[:, opro[name]:opro[name] + width]

        def ce(name, width):
            return cepi_sb[:, oepi[name]:oepi[name] + width]

        w1_sb = cw8_sb
        v_sb = cb16_sb[:, 512:514]
        bS_sb = cp("bS", 2)
        w2_sb = cp("w2", 512)
        decT_sb = cp("decT", B)
        hA_sb = cp("hA", 2 * B)
        aw_ih_sb = cp("aw_ih", 768)
        aw_hh_sb = cp("aw_hh", 1536)
        a_brz_sb = cp("a_brz", 4)
        a_bin_sb = cp("a_bin", 2)
        a_bhn_sb = cp("a_bhn", 2)
        onc_sb = cp("ones", 1)
        onr_sb = cpro_sb[0:1, opro["ones"]:opro["ones"] + 128]
        h1_sb = ce("h1", 2 * B)
        h2_sb = ce("h2", 2 * B)
        g1_ih_sb = ce("g1_ih", 1536)
        g1_hh_sb = ce("g1_hh", 1536)
        g1_brz_sb = ce("g1_brz", 4)
        g1_bin_sb = ce("g1_bin", 2)
        g1_bhn_sb = ce("g1_bhn", 2)
        g2_ih_sb = ce("g2_ih", 1536)
        g2_hh_sb = ce("g2_hh", 1536)
        g2_brz_sb = ce("g2_brz", 4)
        g2_bin_sb = ce("g2_bin", 2)
        g2_bhn_sb = ce("g2_bhn", 2)
        pw_sb = ce("pw", 1024)
        pb_sb = ce("pb", 2)
        ow_sb = ce("ow", 800)
        ob_sb = ce("ob", 4)

        # ---------------- prologue: attention GRU + score bias ------------
        with tc.tile_pool(name="ps_pro", bufs=2, space="PSUM") as ps_pro:
            dtT = _gru(nc, tc, B, "ga", [decT_sb], hA_sb, aw_ih_sb, aw_hh_sb,
                       1, a_brz_sb, a_bin_sb, a_bhn_sb, ps_pro, sm, consts)
            for hb in range(2):
                nc.sync.dma_start(d["o_hA"].ap()[:, hb * B:(hb + 1) * B],
                                  dtT[hb][:])

            biasS_sb = consts.tile([128, 2 * B], f32, tag="biasS",
                                   name="biasS_sb")
            for hb in range(2):
                ps_bs = ps_pro.tile([128, 512], f32, tag="g", name=f"ps_bs{hb}")
                for kt in range(2):
                    nc.tensor.matmul(
                        ps_bs[:, 0:B],
                        w2_sb[:, kt * 256 + hb * 128: kt * 256 + hb * 128 + 128],
                        dtT[kt][:],
                        start=(kt == 0), stop=(kt == 1))
                nc.vector.tensor_scalar_add(
                    biasS_sb[:, hb * B:(hb + 1) * B], ps_bs[:, 0:B],
                    bS_sb[:, hb:hb + 1])

        # ---------------- main streaming loop over batches ----------------
        e_sb = acc.tile([128, 8 * B], bf16, tag="e", name="e_sb")
        zp_sb = acc.tile([128, B], f32, tag="zp", name="zp_sb")
        ps_d = ps_dq.tile([128, 128], f32, tag="dacc", name="ps_d")

        with tc.tile_pool(name="ps_s2", bufs=2, space="PSUM") as ps_s2, \
             tc.tile_pool(name="ps_uq", bufs=1, space="PSUM") as ps_uq:
            ps_u = [ps_uq.tile([128, 256], f32, tag=f"u{i}", name=f"ps_u{i}")
                    for i in range(2)]

            tanh_tiles = {}
            for i in range(B + 3):
                if i < B:
                    b = i
                    if b % CHUNK == 0:
                        c = b // CHUNK + 4
                        if c * CHUNK < B:
                            load_chunk(c)
                    encT_sb = encT_tiles[b // CHUNK]
                    boff = (b % CHUNK) * 2048
                    th = p_tanh.tile([128, 2048], bf16, tag="tanh",
                                     name=f"tanh{b}")
                    tanh_tiles[b] = th
                    for hb in range(2):
                        ps_s = ps_s2.tile([128, 1024], f32, tag="s2",
                                          name=f"psS{b}_{hb}")
                        for kt in range(2):
                            for t2 in range(2):
                                nc.tensor.matmul(
                                    ps_s[:, t2 * 512:(t2 + 1) * 512],
                                    w1_sb[:, kt * 256 + hb * 128:
                                          kt * 256 + hb * 128 + 128],
                                    encT_sb[:, boff + kt * 1024 + t2 * 512:
                                            boff + kt * 1024 + t2 * 512 + 512],
                                    start=(kt == 0), stop=(kt == 1),
                                    skip_group_check=True)
                        nc.scalar.activation(
                            th[:, hb * 1024:(hb + 1) * 1024],
                            ps_s[:], AF.Tanh,
                            bias=biasS_sb[:, hb * B + b: hb * B + b + 1])
                if 2 <= i <= B + 1:
                    b = i - 2
                    th = tanh_tiles.pop(b)
                    pu = ps_u[b % 2]
                    base = (b // 2) * 8
                    for k in range(8):
                        for hb in range(2):
                            nc.tensor.matmul(
                                pu[:, base + k: base + k + 1],
                                th[:, hb * 1024 + k * 128:
                                   hb * 1024 + k * 128 + 128],
                                v_sb[:, hb:hb + 1],
                                start=(hb == 0), stop=(hb == 1))
                    nc.scalar.activation(e_sb[:, b * 8:(b + 1) * 8],
                                         pu[:, base:base + 8], AF.Exp)
                    nc.vector.reduce_sum(zp_sb[:, b:b + 1],
                                         e_sb[:, b * 8:b * 8 + 8], axis=AX.X)
                if 3 <= i <= B + 2:
                    b = i - 3
                    encN_sb = encN_tiles[b // CHUNK]
                    boff = (b % CHUNK) * 2048
                    for hb in range(2):
                        for k in range(8):
                            nc.tensor.matmul(
                                ps_d[:, hb * B + b: hb * B + b + 1],
                                encN_sb[:, boff + k * 256 + hb * 128:
                                        boff + k * 256 + hb * 128 + 128],
                                e_sb[:, b * 8 + k: b * 8 + k + 1],
                                start=(k == 0), stop=(k == 7))
                    if b % CHUNK == CHUNK - 1:
                        encN_tiles.pop(b // CHUNK)
                        if b >= CHUNK:
                            encT_tiles.pop(b // CHUNK - 1, None)
            encT_tiles.clear()
            encN_tiles.clear()

        # ---------------- epilogue ----------------
        with tc.tile_pool(name="ps_epi", bufs=2, space="PSUM") as ps_epi:
            # softmax normalization of d_t_dot
            ps_z = ps_epi.tile([128, 512], f32, tag="g", name="ps_z")
            nc.tensor.matmul(ps_z[0:1, 0:B], onc_sb[:], zp_sb[:],
                             start=True, stop=True)
            rZ_sb = sm.tile([1, B], f32, tag="rZ", name="rZ_sb")
            nc.vector.reciprocal(rZ_sb[:], ps_z[0:1, 0:B])
            ps_r = ps_epi.tile([128, 512], f32, tag="g", name="ps_r")
            nc.tensor.matmul(ps_r[:, 0:B], onr_sb[:], rZ_sb[:],
                             start=True, stop=True)
            rrep_sb = sm.tile([128, B], f32, tag="rrep", name="rrep_sb")
            nc.vector.tensor_copy(rrep_sb[:], ps_r[:, 0:B])
            dtd = []
            for hb in range(2):
                dt_t = consts.tile([128, B], f32, tag=f"dtd{hb}",
                                   name=f"dtd{hb}")
                nc.vector.tensor_mul(dt_t[:], ps_d[:, hb * B:(hb + 1) * B],
                                     rrep_sb[:])
                dtd.append(dt_t)

            # projection
            concat_blocks = [dtT[0], dtT[1], dtd[0], dtd[1]]
            projT = []
            for mb in range(2):
                ps_p = ps_epi.tile([128, 512], f32, tag="g", name=f"ps_p{mb}")
                for kt in range(4):
                    nc.tensor.matmul(
                        ps_p[:, 0:B],
                        pw_sb[:, kt * 256 + mb * 128: kt * 256 + mb * 128 + 128],
                        concat_blocks[kt][:],
                        start=(kt == 0), stop=(kt == 3))
                pj = consts.tile([128, B], f32, tag=f"projT{mb}",
                                 name=f"projT{mb}")
                nc.vector.tensor_scalar_add(pj[:], ps_p[:, 0:B],
                                            pb_sb[:, mb:mb + 1])
                projT.append(pj)

            # GRU1 / GRU2 / output
            g1out = _gru(nc, tc, B, "g1", projT, h1_sb, g1_ih_sb, g1_hh_sb, 2,
                         g1_brz_sb, g1_bin_sb, g1_bhn_sb, ps_epi, sm, consts)
            for hb in range(2):
                nc.sync.dma_start(d["o_h1"].ap()[:, hb * B:(hb + 1) * B],
                                  g1out[hb][:])

            in2 = []
            for hb in range(2):
                t = consts.tile([128, B], f32, tag=f"in2_{hb}",
                                name=f"in2_{hb}")
                nc.vector.tensor_add(t[:], g1out[hb][:], projT[hb][:])
                in2.append(t)

            g2out = _gru(nc, tc, B, "g2", in2, h2_sb, g2_ih_sb, g2_hh_sb, 2,
                         g2_brz_sb, g2_bin_sb, g2_bhn_sb, ps_epi, sm, consts)
            for hb in range(2):
                nc.sync.dma_start(d["o_h2"].ap()[:, hb * B:(hb + 1) * B],
                                  g2out[hb][:])

            sum2 = []
            for hb in range(2):
                t = consts.tile([128, B], f32, tag=f"sum2_{hb}",
                                name=f"sum2_{hb}")
                nc.vector.tensor_add(t[:], in2[hb][:], g2out[hb][:])
                sum2.append(t)

            for mb in range(4):
                mw = min(128, MOUT - mb * 128)
                ps_o = ps_epi.tile([128, 512], f32, tag="g", name=f"ps_o{mb}")
                for kt in range(2):
                    nc.tensor.matmul(
                        ps_o[0:mw, 0:B],
                        ow_sb[:, kt * MOUT + mb * 128: kt * MOUT + mb * 128 + mw],
                        sum2[kt][:],
                        start=(kt == 0), stop=(kt == 1))
                strip = sm.tile([128, B], f32, tag="sm_o", bufs=2,
                                name=f"mel{mb}")
                nc.vector.tensor_scalar_add(strip[0:mw, :], ps_o[0:mw, 0:B],
                                            ob_sb[0:mw, mb:mb + 1])
                nc.sync.dma_start(d["o_mel"].ap()[0:mw, mb * B:(mb + 1) * B],
                                  strip[0:mw, :])


# --------------------------------------------------------------------------
# Host side
# --------------------------------------------------------------------------

def _pblk(x):
    """[K, F] -> [128, (K//128)*F] partition-major block layout."""
    x = np.ascontiguousarray(x)
    K, F = x.shape
    Kt = K // 128
    return np.ascontiguousarray(
        x.reshape(Kt, 128, F).transpose(1, 0, 2)).reshape(128, Kt * F)


def _prep_shared(inp, B):
    """cpro/cepi template blobs (per-core state slots zero) + cb16/cw8."""
    g = lambda k: np.asarray(inp[k], dtype=F32)
    opro, protot = _offsets(_pro_layout(B))
    oepi, epitot = _offsets(_epi_layout(B))
    pro = np.zeros((128, protot), dtype=F32)
    epi = np.zeros((128, epitot), dtype=F32)

    def putp(name, arr):
        pro[:, opro[name]:opro[name] + arr.shape[1]] = arr

    def pute(name, arr):
        epi[:, oepi[name]:oepi[name] + arr.shape[1]] = arr

    putp("bS", (g("W1_b") + g("W2_b")).reshape(2, 128).T)
    putp("w2", _pblk(g("W2_w").T))
    putp("aw_ih", _pblk(g("agru_wih").T))
    putp("aw_hh", _pblk(g("agru_whh").T))
    ab_ih, ab_hh = g("agru_bih"), g("agru_bhh")
    putp("a_brz", (ab_ih + ab_hh)[:512].reshape(4, 128).T)
    putp("a_bin", ab_ih[512:].reshape(2, 128).T)
    putp("a_bhn", ab_hh[512:].reshape(2, 128).T)
    putp("ones", np.ones((128, 128), dtype=F32))
    for pfx, wih, whh, bih, bhh in (
            ("g1", "gru1_wih", "gru1_whh", "gru1_bih", "gru1_bhh"),
            ("g2", "gru2_wih", "gru2_whh", "gru2_bih", "gru2_bhh")):
        pute(f"{pfx}_ih", _pblk(g(wih).T))
        pute(f"{pfx}_hh", _pblk(g(whh).T))
        bih_v, bhh_v = g(bih), g(bhh)
        pute(f"{pfx}_brz", (bih_v + bhh_v)[:512].reshape(4, 128).T)
        pute(f"{pfx}_bin", bih_v[512:].reshape(2, 128).T)
        pute(f"{pfx}_bhn", bhh_v[512:].reshape(2, 128).T)
    pute("pw", _pblk(g("proj_w").T))
    pute("pb", g("proj_b").reshape(2, 128).T)
    pute("ow", _pblk(g("out_w").T))
    ob = np.zeros(512, dtype=F32)
    ob[:MOUT] = g("out_b")
    pute("ob", ob.reshape(4, 128).T)

    cb16 = np.zeros((128, 514), dtype=BF16)
    cb16[:, 0:512] = _pblk(g("W1_w").T).astype(BF16)
    cb16[:, 512:514] = g("V_w").reshape(2, 128).T.astype(BF16)
    cw8 = _pblk(g("W1_w").T).astype(_enct_np())
    return pro, epi, cb16, cw8, opro, oepi


def _make_in_maps(inputs, ncores, B):
    enc = np.asarray(inputs["enc_vec"], dtype=F32)
    dec = np.asarray(inputs["dec_vec"], dtype=F32)
    hA = np.asarray(inputs["atten_GRU_h"], dtype=F32)
    h1 = np.asarray(inputs["gru1_h"], dtype=F32)
    h2 = np.asarray(inputs["gru2_h"], dtype=F32)

    pro0, epi0, cb16, cw8, opro, oepi = _prep_shared(inputs, B)
    in_maps = []
    for c in range(ncores):
        sl = slice(c * B, (c + 1) * B)
        encs = enc[sl]
        encT = encs.transpose(0, 2, 1).reshape(B, 2, 128, T) \
            .transpose(0, 2, 1, 3).reshape(B, 128, 2048).astype(_enct_np())
        encN = encs.reshape(B, 8, 128, H).transpose(0, 2, 1, 3) \
            .reshape(B, 128, 2048).astype(_encn_np())
        pro = pro0.copy()
        pro[:, opro["decT"]:opro["decT"] + B] = dec[sl, 0, :].T
        pro[:, opro["hA"]:opro["hA"] + 2 * B] = _pblk(hA[0, sl].T)
        epi = epi0.copy()
        epi[:, oepi["h1"]:oepi["h1"] + 2 * B] = _pblk(h1[0, sl].T)
        epi[:, oepi["h2"]:oepi["h2"] + 2 * B] = _pblk(h2[0, sl].T)
        m = {
            "encT": np.ascontiguousarray(encT),
            "encN": np.ascontiguousarray(encN),
            "cpro": pro,
            "cepi": epi,
            "cb16": cb16,
            "cw8": cw8,
        }
        in_maps.append(m)
    return in_maps


def _assemble(results, ncores, B):
    outs, hAs, h1s, h2s = [], [], [], []
    for c in range(ncores):
        r = results[c]
        mel = np.asarray(r["o_mel"], dtype=F32)  # [128, 4B]
        mel = mel.reshape(128, 4, B).transpose(1, 0, 2).reshape(512, B)
        outs.append(mel[:MOUT].T.reshape(B, RF, NMEL))

        def unb(x):  # [128, 2B] -> [B, 256]
            x = np.asarray(x, dtype=F32)
            return x.reshape(128, 2, B).transpose(1, 0, 2).reshape(256, B).T

        hAs.append(unb(r["o_hA"]))
        h1s.append(unb(r["o_h1"]))
        h2s.append(unb(r["o_h2"]))

    out = np.concatenate(outs, axis=0)
    next_hA = np.concatenate(hAs, axis=0)[None]
    next_h1 = np.concatenate(h1s, axis=0)[None]
    next_h2 = np.concatenate(h2s, axis=0)[None]
    return out, next_hA, next_h1, next_h2


def kernel(**inputs):
    B = BS // NCORES
    if "nc" not in _CACHE:
        _CACHE["nc"] = _build_nc(B)
    nc = _CACHE["nc"]

    in_maps = _make_in_maps(inputs, NCORES, B)
    trace = os.environ.get("KERNEL_TRACE") == "1"
    res = bass_utils.run_bass_kernel_spmd(
        nc, in_maps, core_ids=list(range(NCORES)), trace=trace)
    _CACHE["last_results"] = res
    return _assemble(res.results, NCORES, B)


# revision 38
# speedup vs baseline: 26.0904x; 1.0103x over previous
"""Trainium2 Bass kernel for nn_Attention_Decoder (bs=512, T=1024, H=256).

Sharding: data-parallel over batch across 8 NeuronCores (64 batches/core).
enc_vec is uploaded twice in fp8e4m3: transposed [h, t] layout (moving
operand of the big W1 @ enc matmul, computed as one DoubleRow K=256 fp8
matmul per output tile) and natural [t, h] layout (stationary operand of
the softmax-weighted-sum matmuls; the bf16 exp-weights are the moving
side). Weights are host-pre-transposed into lhsT block layout; softmax
skips max-subtraction (logits bounded) and folds 1/Z in after the
weighted-sum accumulation.
"""

import os

import numpy as np
import ml_dtypes

import concourse.bacc as bacc
import concourse.tile as tile
from concourse import mybir, bass_utils

BF16 = ml_dtypes.bfloat16
FP8 = ml_dtypes.float8_e4m3
F32 = np.float32

H = 256
T = 1024
BS = 512
NCORES = 8
NMEL, RF = 80, 5
MOUT = NMEL * RF  # 400
ENCN_FP8 = True
ENCT_FP8 = True
S_DOUBLEROW = True
CHUNK = 4  # batches per enc DMA

dt = mybir.dt
AF = mybir.ActivationFunctionType
AX = mybir.AxisListType

_CACHE = {}


def _pro_layout(B):
    """Prologue-critical f32 constants (needed in the first ~10us)."""
    return [
        ("bS", 2), ("w2", 512), ("decT", B), ("hA", 2 * B), ("aw_ih", 768),
        ("aw_hh", 1536), ("a_brz", 4), ("a_bin", 2), ("a_bhn", 2),
        ("ones", 128),
    ]


def _epi_layout(B):
    """Epilogue f32 constants (needed only after the main loop)."""
    return [
        ("h1", 2 * B), ("h2", 2 * B), ("g1_ih", 1536), ("g1_hh", 1536),
        ("g1_brz", 4), ("g1_bin", 2), ("g1_bhn", 2), ("g2_ih", 1536),
        ("g2_hh", 1536), ("g2_brz", 4), ("g2_bin", 2), ("g2_bhn", 2),
        ("pw", 1024), ("pb", 2), ("ow", 800), ("ob", 4),
    ]


def _offsets(layout):
    off, o = {}, 0
    for name, w in layout:
        off[name] = o
        o += w
    return off, o


def _encn_dt():
    return dt.float8e4 if ENCN_FP8 else dt.bfloat16


def _encn_np():
    return FP8 if ENCN_FP8 else BF16


def _enct_dt():
    return dt.float8e4 if ENCT_FP8 else dt.bfloat16


def _enct_np():
    return FP8 if ENCT_FP8 else BF16


# --------------------------------------------------------------------------
# Device program
# --------------------------------------------------------------------------

def _build_nc(B):
    """Build + compile the per-core Bass program for B batches per core."""
    nc = bacc.Bacc("TRN2", target_bir_lowering=False, debug=False)

    f32, bf16 = dt.float32, dt.bfloat16
    d = {}

    def inp(name, shape, dtype):
        d[name] = nc.dram_tensor(name, shape, dtype, kind="ExternalInput")

    def outp(name, shape, dtype):
        d[name] = nc.dram_tensor(name, shape, dtype, kind="ExternalOutput")

    _, protot = _offsets(_pro_layout(B))
    _, epitot = _offsets(_epi_layout(B))
    inp("encT", [B, 128, 2048], _enct_dt())
    inp("encN", [B, 128, 2048], _encn_dt())
    inp("cpro", [128, protot], f32)
    inp("cepi", [128, epitot], f32)
    inp("cb16", [128, 514], bf16)
    inp("cw8", [128, 512], _enct_dt())

    outp("o_mel", [128, 4 * B], f32)
    outp("o_hA", [128, 2 * B], f32)
    outp("o_h1", [128, 2 * B], f32)
    outp("o_h2", [128, 2 * B], f32)

    with tile.TileContext(nc) as tc:
        _emit(nc, tc, B, d)
    nc.compile()
    return nc


def _gru(nc, tc, B, name, x_blocks, h_sb_ap, wih_sb, whh_sb, ktx,
         brz, bin_, bhn, ps_pool, sm, out_pool):
    """One GRU step on transposed activations [feat, B]. h_sb_ap is a
    [128, 2B] tile; returns two [128, B] hout tiles from out_pool."""
    f32 = dt.float32
    rz = []
    for gb in range(4):
        ps_g = ps_pool.tile([128, 512], f32, tag="g", name=f"{name}_psg{gb}")
        nmm = ktx + 2
        i = 0
        for kt in range(ktx):
            nc.tensor.matmul(
                ps_g[:, 0:B],
                wih_sb[:, kt * 768 + gb * 128: kt * 768 + gb * 128 + 128],
                x_blocks[kt][:],
                start=(i == 0), stop=(i == nmm - 1))
            i += 1
        for kt in range(2):
            nc.tensor.matmul(
                ps_g[:, 0:B],
                whh_sb[:, kt * 768 + gb * 128: kt * 768 + gb * 128 + 128],
                h_sb_ap[:, kt * B:(kt + 1) * B],
                start=(i == 0), stop=(i == nmm - 1))
            i += 1
        g_sb = sm.tile([128, B], f32, tag=f"{name}_g{gb}", name=f"{name}_g{gb}")
        nc.scalar.activation(g_sb[:], ps_g[:, 0:B], AF.Sigmoid,
                             bias=brz[:, gb:gb + 1])
        rz.append(g_sb)
    houts = []
    for nb in range(2):
        gb = 4 + nb
        ps_i = ps_pool.tile([128, 512], f32, tag="g", name=f"{name}_psi{nb}")
        for kt in range(ktx):
            nc.tensor.matmul(
                ps_i[:, 0:B],
                wih_sb[:, kt * 768 + gb * 128: kt * 768 + gb * 128 + 128],
                x_blocks[kt][:],
                start=(kt == 0), stop=(kt == ktx - 1))
        ps_h = ps_pool.tile([128, 512], f32, tag="g", name=f"{name}_psh{nb}")
        for kt in range(2):
            nc.tensor.matmul(
                ps_h[:, 0:B],
                whh_sb[:, kt * 768 + gb * 128: kt * 768 + gb * 128 + 128],
                h_sb_ap[:, kt * B:(kt + 1) * B],
                start=(kt == 0), stop=(kt == 1))
        hnb = sm.tile([128, B], f32, tag=f"{name}_hnb{nb}", name=f"{name}_hnb{nb}")
        nc.vector.tensor_scalar_add(hnb[:], ps_h[:, 0:B], bhn[:, nb:nb + 1])
        rhn = sm.tile([128, B], f32, tag=f"{name}_rhn{nb}", name=f"{name}_rhn{nb}")
        nc.vector.tensor_mul(rhn[:], rz[nb][:], hnb[:])
        t1 = sm.tile([128, B], f32, tag=f"{name}_t1{nb}", name=f"{name}_t1{nb}")
        nc.vector.tensor_add(t1[:], ps_i[:, 0:B], rhn[:])
        n_sb = sm.tile([128, B], f32, tag=f"{name}_n{nb}", name=f"{name}_n{nb}")
        nc.scalar.activation(n_sb[:], t1[:], AF.Tanh, bias=bin_[:, nb:nb + 1])
        dd = sm.tile([128, B], f32, tag=f"{name}_d{nb}", name=f"{name}_d{nb}")
        nc.vector.tensor_sub(dd[:], h_sb_ap[:, nb * B:(nb + 1) * B], n_sb[:])
        zd = sm.tile([128, B], f32, tag=f"{name}_zd{nb}", name=f"{name}_zd{nb}")
        nc.vector.tensor_mul(zd[:], rz[2 + nb][:], dd[:])
        ho = out_pool.tile([128, B], f32, tag=f"{name}_h{nb}", name=f"{name}_h{nb}")
        nc.vector.tensor_add(ho[:], n_sb[:], zd[:])
        houts.append(ho)
    return houts


def _emit(nc, tc, B, d):
    f32, bf16 = dt.float32, dt.bfloat16
    endt = _encn_dt()
    from contextlib import ExitStack

    assert B % (2 * CHUNK) == 0
    nchunks = B // CHUNK

    with ExitStack() as ctx:
        consts = ctx.enter_context(tc.tile_pool(name="consts", bufs=1))
        acc = ctx.enter_context(tc.tile_pool(name="acc", bufs=1))
        sm = ctx.enter_context(tc.tile_pool(name="sm", bufs=1))
        p_encT = ctx.enter_context(tc.tile_pool(name="p_encT", bufs=5))
        p_encN = ctx.enter_context(tc.tile_pool(name="p_encN", bufs=5))
        p_tanh = ctx.enter_context(tc.tile_pool(name="p_tanh", bufs=3))
        # PSUM pools: ps_dq (1 bank) lives from main loop into the epilogue.
        ps_dq = ctx.enter_context(tc.tile_pool(name="ps_dq", bufs=1, space="PSUM"))

        # hoist the first enc chunk DMAs ahead of everything else so the
        # tensor engine has work as soon as possible
        encT_ap0 = d["encT"].ap()
        encN_ap0 = d["encN"].ap()
        encT_tiles = {}
        encN_tiles = {}

        def load_chunk(c):
            eT = p_encT.tile([128, CHUNK * 2048], _enct_dt(), tag="encT",
                             name=f"encTc{c}")
            nc.sync.dma_start(
                eT[:],
                encT_ap0[c * CHUNK:(c + 1) * CHUNK].rearrange("b p x -> p b x"))
            encT_tiles[c] = eT
            eN = p_encN.tile([128, CHUNK * 2048], _encn_dt(), tag="encN",
                             name=f"encNc{c}")
            nc.gpsimd.dma_start(
                eN[:],
                encN_ap0[c * CHUNK:(c + 1) * CHUNK].rearrange("b p x -> p b x"))
            encN_tiles[c] = eN

        # packed consts: prologue blob first (small), epilogue blob later
        opro, protot = _offsets(_pro_layout(B))
        oepi, epitot = _offsets(_epi_layout(B))
        cpro_sb = consts.tile([128, protot], dt.float32, tag="cpro",
                              name="cpro_sb")
        nc.gpsimd.dma_start(cpro_sb[:], d["cpro"].ap()[:])
        cb16_sb = consts.tile([128, 514], bf16, tag="cb16", name="cb16_sb")
        nc.gpsimd.dma_start(cb16_sb[:], d["cb16"].ap()[:])
        cw8_sb = consts.tile([128, 512], _enct_dt(), tag="cw8", name="cw8_sb")
        nc.gpsimd.dma_start(cw8_sb[:], d["cw8"].ap()[:])

        load_chunk(0)
        load_chunk(1)
        load_chunk(2)
        load_chunk(3)

        # epilogue consts: big blob, needed only ~150us in
        cepi_sb = consts.tile([128, epitot], dt.float32, tag="cepi",
                              name="cepi_sb")
        nc.gpsimd.dma_start(cepi_sb[:], d["cepi"].ap()[:])

        def cp(name, width):
            return cpro_sb[:, opro[name]:opro[name] + width]

        def ce(name, width):
            return cepi_sb[:, oepi[name]:oepi[name] + width]

        w1_sb = cw8_sb
        v_sb = cb16_sb[:, 512:514]
        bS_sb = cp("bS", 2)
        w2_sb = cp("w2", 512)
        decT_sb = cp("decT", B)
        hA_sb = cp("hA", 2 * B)
        aw_ih_sb = cp("aw_ih", 768)
        aw_hh_sb = cp("aw_hh", 1536)
        a_brz_sb = cp("a_brz", 4)
        a_bin_sb = cp("a_bin", 2)
        a_bhn_sb = cp("a_bhn", 2)
        onc_sb = cp("ones", 1)
        onr_sb = cpro_sb[0:1, opro["ones"]:opro["ones"] + 128]
        h1_sb = ce("h1", 2 * B)
        h2_sb = ce("h2", 2 * B)
        g1_ih_sb = ce("g1_ih", 1536)
        g1_hh_sb = ce("g1_hh", 1536)
        g1_brz_sb = ce("g1_brz", 4)
        g1_bin_sb = ce("g1_bin", 2)
        g1_bhn_sb = ce("g1_bhn", 2)
        g2_ih_sb = ce("g2_ih", 1536)
        g2_hh_sb = ce("g2_hh", 1536)
        g2_brz_sb = ce("g2_brz", 4)
        g2_bin_sb = ce("g2_bin", 2)
        g2_bhn_sb = ce("g2_bhn", 2)
        pw_sb = ce("pw", 1024)
        pb_sb = ce("pb", 2)
        ow_sb = ce("ow", 800)
        ob_sb = ce("ob", 4)

        # ---------------- prologue: attention GRU + score bias ------------
        with tc.tile_pool(name="ps_pro", bufs=2, space="PSUM") as ps_pro:
            dtT = _gru(nc, tc, B, "ga", [decT_sb], hA_sb, aw_ih_sb, aw_hh_sb,
                       1, a_brz_sb, a_bin_sb, a_bhn_sb, ps_pro, sm, consts)
            for hb in range(2):
                nc.sync.dma_start(d["o_hA"].ap()[:, hb * B:(hb + 1) * B],
                                  dtT[hb][:])

            biasS_sb = consts.tile([128, 2 * B], f32, tag="biasS",
                                   name="biasS_sb")
            for hb in range(2):
                ps_bs = ps_pro.tile([128, 512], f32, tag="g", name=f"ps_bs{hb}")
                for kt in range(2):
                    nc.tensor.matmul(
                        ps_bs[:, 0:B],
                        w2_sb[:, kt * 256 + hb * 128: kt * 256 + hb * 128 + 128],
                        dtT[kt][:],
                        start=(kt == 0), stop=(kt == 1))
                nc.vector.tensor_scalar_add(
                    biasS_sb[:, hb * B:(hb + 1) * B], ps_bs[:, 0:B],
                    bS_sb[:, hb:hb + 1])

        # ---------------- main streaming loop over batches ----------------
        e_sb = acc.tile([128, 8 * B], bf16, tag="e", name="e_sb")
        zp_sb = acc.tile([128, B], f32, tag="zp", name="zp_sb")
        ps_d = ps_dq.tile([128, 128], f32, tag="dacc", name="ps_d")

        with tc.tile_pool(name="ps_s2", bufs=2, space="PSUM") as ps_s2, \
             tc.tile_pool(name="ps_uq", bufs=1, space="PSUM") as ps_uq:
            ps_u = [ps_uq.tile([128, 256], f32, tag=f"u{i}", name=f"ps_u{i}")
                    for i in range(2)]

            tanh_tiles = {}
            for i in range(B + 3):
                if i < B:
                    b = i
                    if b % CHUNK == 0:
                        c = b // CHUNK + 4
                        if c * CHUNK < B:
                            load_chunk(c)
                    encT_sb = encT_tiles[b // CHUNK]
                    boff = (b % CHUNK) * 2048
                    th = p_tanh.tile([128, 2048], bf16, tag="tanh",
                                     name=f"tanh{b}")
                    tanh_tiles[b] = th
                    encT_3d = encT_sb.rearrange(
                        "p (bc kt t) -> p bc kt t", bc=CHUNK, kt=2)
                    w1_3d = w1_sb.rearrange("p (kt f) -> p kt f", kt=2)
                    for hb in range(2):
                        ps_s = ps_s2.tile([128, 1024], f32, tag="s2",
                                          name=f"psS{b}_{hb}")
                        if S_DOUBLEROW and ENCT_FP8:
                            for t2 in range(2):
                                nc.tensor.matmul(
                                    ps_s[:, t2 * 512:(t2 + 1) * 512],
                                    w1_3d[:, :, hb * 128:(hb + 1) * 128],
                                    encT_3d[:, b % CHUNK, :,
                                            t2 * 512:(t2 + 1) * 512],
                                    start=True, stop=True,
                                    perf_mode=mybir.MatmulPerfMode.DoubleRow,
                                    skip_group_check=True)
                        else:
                            for kt in range(2):
                                for t2 in range(2):
                                    nc.tensor.matmul(
                                        ps_s[:, t2 * 512:(t2 + 1) * 512],
                                        w1_sb[:, kt * 256 + hb * 128:
                                              kt * 256 + hb * 128 + 128],
                                        encT_sb[:, boff + kt * 1024 + t2 * 512:
                                                boff + kt * 1024 + t2 * 512 + 512],
                                        start=(kt == 0), stop=(kt == 1),
                                        skip_group_check=True)
                        nc.scalar.activation(
                            th[:, hb * 1024:(hb + 1) * 1024],
                            ps_s[:], AF.Tanh,
                            bias=biasS_sb[:, hb * B + b: hb * B + b + 1])
                if 2 <= i <= B + 1:
                    b = i - 2
                    th = tanh_tiles.pop(b)
                    pu = ps_u[b % 2]
                    base = (b // 2) * 8
                    for k in range(8):
                        for hb in range(2):
                            nc.tensor.matmul(
                                pu[:, base + k: base + k + 1],
                                th[:, hb * 1024 + k * 128:
                                   hb * 1024 + k * 128 + 128],
                                v_sb[:, hb:hb + 1],
                                start=(hb == 0), stop=(hb == 1))
                    nc.scalar.activation(e_sb[:, b * 8:(b + 1) * 8],
                                         pu[:, base:base + 8], AF.Exp)
                    nc.vector.reduce_sum(zp_sb[:, b:b + 1],
                                         e_sb[:, b * 8:b * 8 + 8], axis=AX.X)
                if 3 <= i <= B + 2:
                    b = i - 3
                    encN_sb = encN_tiles[b // CHUNK]
                    boff = (b % CHUNK) * 2048
                    for hb in range(2):
                        for k in range(8):
                            nc.tensor.matmul(
                                ps_d[:, hb * B + b: hb * B + b + 1],
                                encN_sb[:, boff + k * 256 + hb * 128:
                                        boff + k * 256 + hb * 128 + 128],
                                e_sb[:, b * 8 + k: b * 8 + k + 1],
                                start=(k == 0), stop=(k == 7))
                    if b % CHUNK == CHUNK - 1:
                        encN_tiles.pop(b // CHUNK)
                        if b >= CHUNK:
                            encT_tiles.pop(b // CHUNK - 1, None)
            encT_tiles.clear()
            encN_tiles.clear()

        # ---------------- epilogue ----------------
        with tc.tile_pool(name="ps_epi", bufs=2, space="PSUM") as ps_epi:
            # softmax normalization of d_t_dot
            ps_z = ps_epi.tile([128, 512], f32, tag="g", name="ps_z")
            nc.tensor.matmul(ps_z[0:1, 0:B], onc_sb[:], zp_sb[:],
                             start=True, stop=True)
            rZ_sb = sm.tile([1, B], f32, tag="rZ", name="rZ_sb")
            nc.vector.reciprocal(rZ_sb[:], ps_z[0:1, 0:B])
            ps_r = ps_epi.tile([128, 512], f32, tag="g", name="ps_r")
            nc.tensor.matmul(ps_r[:, 0:B], onr_sb[:], rZ_sb[:],
                             start=True, stop=True)
            rrep_sb = sm.tile([128, B], f32, tag="rrep", name="rrep_sb")
            nc.vector.tensor_copy(rrep_sb[:], ps_r[:, 0:B])
            dtd = []
            for hb in range(2):
                dt_t = consts.tile([128, B], f32, tag=f"dtd{hb}",
                                   name=f"dtd{hb}")
                nc.vector.tensor_mul(dt_t[:], ps_d[:, hb * B:(hb + 1) * B],
                                     rrep_sb[:])
                dtd.append(dt_t)

            # projection
            concat_blocks = [dtT[0], dtT[1], dtd[0], dtd[1]]
            projT = []
            for mb in range(2):
                ps_p = ps_epi.tile([128, 512], f32, tag="g", name=f"ps_p{mb}")
                for kt in range(4):
                    nc.tensor.matmul(
                        ps_p[:, 0:B],
                        pw_sb[:, kt * 256 + mb * 128: kt * 256 + mb * 128 + 128],
                        concat_blocks[kt][:],
                        start=(kt == 0), stop=(kt == 3))
                pj = consts.tile([128, B], f32, tag=f"projT{mb}",
                                 name=f"projT{mb}")
                nc.vector.tensor_scalar_add(pj[:], ps_p[:, 0:B],
                                            pb_sb[:, mb:mb + 1])
                projT.append(pj)

            # GRU1 / GRU2 / output
            g1out = _gru(nc, tc, B, "g1", projT, h1_sb, g1_ih_sb, g1_hh_sb, 2,
                         g1_brz_sb, g1_bin_sb, g1_bhn_sb, ps_epi, sm, consts)
            for hb in range(2):
                nc.sync.dma_start(d["o_h1"].ap()[:, hb * B:(hb + 1) * B],
                                  g1out[hb][:])

            in2 = []
            for hb in range(2):
                t = consts.tile([128, B], f32, tag=f"in2_{hb}",
                                name=f"in2_{hb}")
                nc.vector.tensor_add(t[:], g1out[hb][:], projT[hb][:])
                in2.append(t)

            g2out = _gru(nc, tc, B, "g2", in2, h2_sb, g2_ih_sb, g2_hh_sb, 2,
                         g2_brz_sb, g2_bin_sb, g2_bhn_sb, ps_epi, sm, consts)
            for hb in range(2):
                nc.sync.dma_start(d["o_h2"].ap()[:, hb * B:(hb + 1) * B],
                                  g2out[hb][:])

            sum2 = []
            for hb in range(2):
                t = consts.tile([128, B], f32, tag=f"sum2_{hb}",
                                name=f"sum2_{hb}")
                nc.vector.tensor_add(t[:], in2[hb][:], g2out[hb][:])
                sum2.append(t)

            for mb in range(4):
                mw = min(128, MOUT - mb * 128)
                ps_o = ps_epi.tile([128, 512], f32, tag="g", name=f"ps_o{mb}")
                for kt in range(2):
                    nc.tensor.matmul(
                        ps_o[0:mw, 0:B],
                        ow_sb[:, kt * MOUT + mb * 128: kt * MOUT + mb * 128 + mw],
                        sum2[kt][:],
                        start=(kt == 0), stop=(kt == 1))
                strip = sm.tile([128, B], f32, tag="sm_o", bufs=2,
                                name=f"mel{mb}")
                nc.vector.tensor_scalar_add(strip[0:mw, :], ps_o[0:mw, 0:B],
                                            ob_sb[0:mw, mb:mb + 1])
                nc.sync.dma_start(d["o_mel"].ap()[0:mw, mb * B:(mb + 1) * B],
                                  strip[0:mw, :])


# --------------------------------------------------------------------------
# Host side
# --------------------------------------------------------------------------

def _pblk(x):
    """[K, F] -> [128, (K//128)*F] partition-major block layout."""
    x = np.ascontiguousarray(x)
    K, F = x.shape
    Kt = K // 128
    return np.ascontiguousarray(
        x.reshape(Kt, 128, F).transpose(1, 0, 2)).reshape(128, Kt * F)


def _prep_shared(inp, B):
    """cpro/cepi template blobs (per-core state slots zero) + cb16/cw8."""
    g = lambda k: np.asarray(inp[k], dtype=F32)
    opro, protot = _offsets(_pro_layout(B))
    oepi, epitot = _offsets(_epi_layout(B))
    pro = np.zeros((128, protot), dtype=F32)
    epi = np.zeros((128, epitot), dtype=F32)

    def putp(name, arr):
        pro[:, opro[name]:opro[name] + arr.shape[1]] = arr

    def pute(name, arr):
        epi[:, oepi[name]:oepi[name] + arr.shape[1]] = arr

    putp("bS", (g("W1_b") + g("W2_b")).reshape(2, 128).T)
    putp("w2", _pblk(g("W2_w").T))
    putp("aw_ih", _pblk(g("agru_wih").T))
    putp("aw_hh", _pblk(g("agru_whh").T))
    ab_ih, ab_hh = g("agru_bih"), g("agru_bhh")
    putp("a_brz", (ab_ih + ab_hh)[:512].reshape(4, 128).T)
    putp("a_bin", ab_ih[512:].reshape(2, 128).T)
    putp("a_bhn", ab_hh[512:].reshape(2, 128).T)
    putp("ones", np.ones((128, 128), dtype=F32))
    for pfx, wih, whh, bih, bhh in (
            ("g1", "gru1_wih", "gru1_whh", "gru1_bih", "gru1_bhh"),
            ("g2", "gru2_wih", "gru2_whh", "gru2_bih", "gru2_bhh")):
        pute(f"{pfx}_ih", _pblk(g(wih).T))
        pute(f"{pfx}_hh", _pblk(g(whh).T))
        bih_v, bhh_v = g(bih), g(bhh)
        pute(f"{pfx}_brz", (bih_v + bhh_v)[:512].reshape(4, 128).T)
        pute(f"{pfx}_bin", bih_v[512:].reshape(2, 128).T)
        pute(f"{pfx}_bhn", bhh_v[512:].reshape(2, 128).T)
    pute("pw", _pblk(g("proj_w").T))
    pute("pb", g("proj_b").reshape(2, 128).T)
    pute("ow", _pblk(g("out_w").T))
    ob = np.zeros(512, dtype=F32)
    ob[:MOUT] = g("out_b")
    pute("ob", ob.reshape(4, 128).T)

    cb16 = np.zeros((128, 514), dtype=BF16)
    cb16[:, 0:512] = _pblk(g("W1_w").T).astype(BF16)
    cb16[:, 512:514] = g("V_w").reshape(2, 128).T.astype(BF16)
    cw8 = _pblk(g("W1_w").T).astype(_enct_np())
    return pro, epi, cb16, cw8, opro, oepi


def _make_in_maps(inputs, ncores, B):
    enc = np.asarray(inputs["enc_vec"], dtype=F32)
    dec = np.asarray(inputs["dec_vec"], dtype=F32)
    hA = np.asarray(inputs["atten_GRU_h"], dtype=F32)
    h1 = np.asarray(inputs["gru1_h"], dtype=F32)
    h2 = np.asarray(inputs["gru2_h"], dtype=F32)

    pro0, epi0, cb16, cw8, opro, oepi = _prep_shared(inputs, B)
    in_maps = []
    for c in range(ncores):
        sl = slice(c * B, (c + 1) * B)
        encs = enc[sl]
        encT = encs.transpose(0, 2, 1).reshape(B, 2, 128, T) \
            .transpose(0, 2, 1, 3).reshape(B, 128, 2048).astype(_enct_np())
        encN = encs.reshape(B, 8, 128, H).transpose(0, 2, 1, 3) \
            .reshape(B, 128, 2048).astype(_encn_np())
        pro = pro0.copy()
        pro[:, opro["decT"]:opro["decT"] + B] = dec[sl, 0, :].T
        pro[:, opro["hA"]:opro["hA"] + 2 * B] = _pblk(hA[0, sl].T)
        epi = epi0.copy()
        epi[:, oepi["h1"]:oepi["h1"] + 2 * B] = _pblk(h1[0, sl].T)
        epi[:, oepi["h2"]:oepi["h2"] + 2 * B] = _pblk(h2[0, sl].T)
        m = {
            "encT": np.ascontiguousarray(encT),
            "encN": np.ascontiguousarray(encN),
            "cpro": pro,
            "cepi": epi,
            "cb16": cb16,
            "cw8": cw8,
        }
        in_maps.append(m)
    return in_maps


def _assemble(results, ncores, B):
    outs, hAs, h1s, h2s = [], [], [], []
    for c in range(ncores):
        r = results[c]
        mel = np.asarray(r["o_mel"], dtype=F32)  # [128, 4B]
        mel = mel.reshape(128, 4, B).transpose(1, 0, 2).reshape(512, B)
        outs.append(mel[:MOUT].T.reshape(B, RF, NMEL))

        def unb(x):  # [128, 2B] -> [B, 256]
            x = np.asarray(x, dtype=F32)
            return x.reshape(128, 2, B).transpose(1, 0, 2).reshape(256, B).T

        hAs.append(unb(r["o_hA"]))
        h1s.append(unb(r["o_h1"]))
        h2s.append(unb(r["o_h2"]))

    out = np.concatenate(outs, axis=0)
    next_hA = np.concatenate(hAs, axis=0)[None]
    next_h1 = np.concatenate(h1s, axis=0)[None]
    next_h2 = np.concatenate(h2s, axis=0)[None]
    return out, next_hA, next_h1, next_h2


def kernel(**inputs):
    B = BS // NCORES
    if "nc" not in _CACHE:
        _CACHE["nc"] = _build_nc(B)
    nc = _CACHE["nc"]

    in_maps = _make_in_maps(inputs, NCORES, B)
    trace = os.environ.get("KERNEL_TRACE") == "1"
    res = bass_utils.run_bass_kernel_spmd(
        nc, in_maps, core_ids=list(range(NCORES)), trace=trace)
    _CACHE["last_results"] = res
    return _assemble(res.results, NCORES, B)
